# revision 16
# baseline (speedup 1.0000x reference)
# Trainium2 Bass kernel for nn_AttentionBlock (GroupNorm + single-head
# self-attention over 32x32 spatial, C=512) — data-parallel over batch:
# 8 batch elements -> 8 NeuronCores, weights replicated.
#
# fp8(e4m3) DoubleRow design: all big matmuls run as fp8 DoubleRow
# (256-deep contraction per instruction, ~259ns/MM sustained), scores
# are computed transposed (k stationary) so no PE transposes are
# needed, softmax row-sums come from a ones-matrix matmul broadcast to
# all partitions, and the softmax normalization is folded into the
# attn@V eviction. Scale bookkeeping: weights x64 on host, activations
# stored x4 in fp8, descale factors folded into the PSUM evictions.
# Evictions are split ACT/DVE to balance engine load; GpSimd is not
# used for compute (it is ~25x slower and contends for DVE's SBUF port).
import numpy as np

CH = 512          # channels
N = 1024          # spatial H*W = 32*32
P = 128           # SBUF partitions
KT = CH // P      # 4 channel tiles
MT = N // P       # 8 spatial tiles (keys)
GROUPS = 8        # groupnorm groups (64 channels each)
EPS = 1e-5
SCALE = 1.0 / np.sqrt(CH)
NCORES = 8

_CACHE = {}


def _build_bass():
    import concourse.bacc as bacc
    import concourse.tile as tile
    from concourse import mybir

    f32 = mybir.dt.float32
    f8 = mybir.dt.float8e4
    bf16 = mybir.dt.bfloat16
    Act = mybir.ActivationFunctionType
    Alu = mybir.AluOpType
    DR = mybir.MatmulPerfMode.DoubleRow

    nc = bacc.Bacc("TRN2")

    x_d = nc.dram_tensor("x", [CH, N], bf16, kind="ExternalInput")
    wq_d = nc.dram_tensor("wq8", [P, KT, CH], f8, kind="ExternalInput")
    wk_d = nc.dram_tensor("wk8", [P, KT, CH], f8, kind="ExternalInput")
    wv_d = nc.dram_tensor("wv8", [P, KT, CH], f8, kind="ExternalInput")
    wp_d = nc.dram_tensor("wp8", [P, KT, CH], f8, kind="ExternalInput")
    # packed per-channel vectors (cols 0..19 = 4*bq|4*bk|gnw|gnb|bp')
    # followed by the 128x128 group-averaging matrix (cols 20..147)
    con_d = nc.dram_tensor("consts", [P, 148], f32, kind="ExternalInput")
    y_d = nc.dram_tensor("y", [CH, N], bf16, kind="ExternalOutput")

    with tile.TileContext(nc) as tc:
        with (
            tc.tile_pool(name="persist", bufs=1) as persist,
            tc.tile_pool(name="work", bufs=2) as work,
            tc.tile_pool(name="small", bufs=2) as small,
        ):
            # ---- persistent SBUF tensors ----
            x_sb = persist.tile([P, KT, N], bf16, tag="x")
            n_sb = persist.tile([P, KT, N], f8, tag="n")
            q_sb = persist.tile([P, KT, N], f8, tag="q")
            k_sb = persist.tile([P, KT, N], f8, tag="k")
            vT_sb = persist.tile([P, MT, CH], f8, tag="vT")
            eT_sb = persist.tile([P, MT, N], f8, tag="eT")
            o_sb = persist.tile([P, KT, N], f8, tag="o")
            rsr_sb = persist.tile([P, N], f32, tag="rsr")
            xbp_sb = persist.tile([P, KT, N], f32, tag="xbp")
            wq_sb = persist.tile([P, KT, CH], f8, tag="wq")
            wk_sb = persist.tile([P, KT, CH], f8, tag="wk")
            wv_sb = persist.tile([P, KT, CH], f8, tag="wv")
            wp_sb = persist.tile([P, KT, CH], f8, tag="wp")
            con_sb = persist.tile([P, 148], f32, tag="consts")
            vec_sb = con_sb[:, 0:20]
            avg_sb = persist.tile([P, P], bf16, tag="avg")
            ones_sb = persist.tile([P, 2, P], f8, tag="ones")
            warm_sb = persist.tile([P, 2, P], f8, tag="warm")
            scr_sb = persist.tile([P, N], f32, tag="scr")
            zero_sb = persist.tile([P, 1], f32, tag="zero")
            eps_sb = persist.tile([P, 1], f32, tag="eps")
            dummy_sb = persist.tile([P, 1], f32, tag="dummy")
            bq_sb = vec_sb[:, 0:4]     # 4*q_b
            bk_sb = vec_sb[:, 4:8]     # 4*k_b
            gnw_sb = vec_sb[:, 8:12]
            gnb_sb = vec_sb[:, 12:16]
            bp_sb = vec_sb[:, 16:20]   # p_w @ v_b + p_b

            # constants + ACT sqrt-table preload while DMAs stream
            nc.vector.memset(zero_sb, 0.0)
            nc.vector.memset(eps_sb, EPS)
            nc.vector.memset(dummy_sb, 1.0)
            nc.vector.memset(ones_sb, 1.0)
            nc.vector.memset(warm_sb, 1.0)
            nc.scalar.activation(out=dummy_sb, in_=dummy_sb, func=Act.Sqrt,
                                 bias=zero_sb, scale=1.0)

            # ---- loads: x on the sync queue, weights in parallel on the
            # gpsimd queue (gpsimd does no compute in this kernel) ----
            for h in range(2):
                nc.sync.dma_start(
                    out=x_sb[:, 0, h * 512:(h + 1) * 512],
                    in_=x_d[0:P, h * 512:(h + 1) * 512])
                nc.scalar.dma_start(
                    out=x_sb[:, 1, h * 512:(h + 1) * 512],
                    in_=x_d[P:2 * P, h * 512:(h + 1) * 512])
            nc.sync.dma_start(out=x_sb[:, 2, :], in_=x_d[2 * P:3 * P, :])
            nc.scalar.dma_start(out=x_sb[:, 3, :], in_=x_d[3 * P:4 * P, :])
            nc.gpsimd.dma_start(out=con_sb[:], in_=con_d[:])
            nc.gpsimd.dma_start(out=wq_sb[:], in_=wq_d[:])
            nc.gpsimd.dma_start(out=wk_sb[:], in_=wk_d[:])
            nc.gpsimd.dma_start(out=wv_sb[:], in_=wv_d[:])
            nc.gpsimd.dma_start(out=wp_sb[:], in_=wp_d[:])

            with tc.tile_pool(name="ps_g", bufs=2, space="PSUM") as ps_g:
                # PE warmth: cheap small fp8 DR matmuls into a scratch bank
                warm_ps = ps_g.tile([P, P], f32, tag="warmps")

                def warm(k):
                    for _ in range(k):
                        nc.tensor.matmul(warm_ps, ones_sb[:], warm_sb[:],
                                         start=True, stop=True, perf_mode=DR)

                warm(28)

                # cast the group-averaging matrix to bf16 (1-pass matmuls)
                nc.vector.tensor_copy(avg_sb, con_sb[:, 20:148])

                # ---- GroupNorm, two batches of 2 channel-tiles each:
                # bn_stats (DVE) -> one bf16 group matmul per batch ->
                # short DVE chain -> per-kt affine on ACT -> n in fp8 ----
                for b in range(2):
                    st = small.tile([P, 4], bf16, tag="st")  # m0|m1|E0|E1
                    for j in range(2):
                        kt = 2 * b + j
                        bstats = small.tile([P, 2, 6], f32, tag="bstats")
                        mv = small.tile([P, 2], f32, tag="mv")
                        nc.vector.bn_stats(out=bstats[:, 0, :], in_=x_sb[:, kt, 0:512])
                        nc.vector.bn_stats(out=bstats[:, 1, :], in_=x_sb[:, kt, 512:1024])
                        nc.vector.bn_aggr(out=mv, in_=bstats)
                        nc.vector.tensor_copy(st[:, j:j + 1], mv[:, 0:1])
                        nc.vector.scalar_tensor_tensor(
                            out=st[:, 2 + j:3 + j], in0=mv[:, 0:1], scalar=mv[:, 0:1],
                            in1=mv[:, 1:2], op0=Alu.mult, op1=Alu.add,
                        )

                    # group aggregate+broadcast for both kt in one matmul
                    b_ps = ps_g.tile([P, 4], f32, tag="gmm")
                    nc.tensor.matmul(b_ps, avg_sb, st, start=True, stop=True)
                    warm(12)
                    bc = small.tile([P, 4], f32, tag="bc")
                    nc.vector.tensor_copy(bc, b_ps)
                    gmean = bc[:, 0:2]
                    msq = small.tile([P, 2], f32, tag="msq")
                    nc.vector.tensor_tensor(out=msq, in0=gmean, in1=gmean,
                                            op=Alu.mult)
                    vneg = small.tile([P, 2], f32, tag="vneg")
                    nc.vector.tensor_tensor(out=vneg, in0=msq, in1=bc[:, 2:4],
                                            op=Alu.subtract)  # mean^2 - E
                    sd = small.tile([P, 2], f32, tag="sd")
                    nc.scalar.activation(out=sd, in_=vneg, func=Act.Sqrt,
                                         bias=eps_sb, scale=-1.0)
                    rstd = small.tile([P, 2], f32, tag="rstd")
                    nc.vector.reciprocal(rstd, sd)
                    gsc = small.tile([P, 2], f32, tag="gsc")
                    nc.vector.tensor_tensor(out=gsc, in0=gnw_sb[:, 2 * b:2 * b + 2],
                                            in1=rstd, op=Alu.mult)
                    mg = small.tile([P, 2], f32, tag="mg")
                    nc.vector.tensor_tensor(out=mg, in0=gmean, in1=gsc,
                                            op=Alu.mult)
                    gshp = small.tile([P, 2], f32, tag="gshp")  # gnb - mean*gsc
                    nc.vector.tensor_tensor(out=gshp,
                                            in0=gnb_sb[:, 2 * b:2 * b + 2],
                                            in1=mg, op=Alu.subtract)
                    for j in range(2):
                        kt = 2 * b + j
                        # n8 = x*gsc + gshp  (ACT, fp8 out)
                        nc.scalar.activation(out=n_sb[:, kt, :],
                                             in_=x_sb[:, kt, :],
                                             func=Act.Identity,
                                             bias=gshp[:, j:j + 1],
                                             scale=gsc[:, j:j + 1])

                warm(12)
                # preload the exp table while the QKV matmuls stream
                nc.scalar.activation(out=dummy_sb, in_=x_sb[:, 0, 0:1], func=Act.Exp,
                                     bias=zero_sb, scale=1.0)

            with tc.tile_pool(name="ps_qkv", bufs=4, space="PSUM") as ps_qkv:
                # ---- Q projection (DoubleRow, weights stationary).
                # ktp-outer: the ktp=0 sweep only needs n8 kt0/kt1, so the
                # PE starts while the GN tail still runs on ACT/DVE. ----
                q_mm = [ps_qkv.tile([P, N], f32, tag="mm", name=f"q{dt}")
                        for dt in range(KT)]
                for ktp in range(2):
                    for dt in range(KT):
                        for nh in range(2):
                            nc.tensor.matmul(
                                q_mm[dt][:, nh * 512:(nh + 1) * 512],
                                wq_sb[:, 2 * ktp:2 * ktp + 2, dt * P:(dt + 1) * P],
                                n_sb[:, 2 * ktp:2 * ktp + 2, nh * 512:(nh + 1) * 512],
                                start=(ktp == 0), stop=(ktp == 1), perf_mode=DR,
                            )
                        if ktp == 1:
                            # q8 = raw/16 + 4*bq  (= 4*q_true), ACT evict
                            nc.scalar.activation(out=q_sb[:, dt, :], in_=q_mm[dt],
                                                 func=Act.Identity,
                                                 bias=bq_sb[:, dt:dt + 1],
                                                 scale=1.0 / 16)

                # ---- K projection: evict on DVE to balance engines ----
                k_mm = [ps_qkv.tile([P, N], f32, tag="mm", name=f"k{dt}")
                        for dt in range(KT)]
                for ktp in range(2):
                    for dt in range(KT):
                        for nh in range(2):
                            nc.tensor.matmul(
                                k_mm[dt][:, nh * 512:(nh + 1) * 512],
                                wk_sb[:, 2 * ktp:2 * ktp + 2, dt * P:(dt + 1) * P],
                                n_sb[:, 2 * ktp:2 * ktp + 2, nh * 512:(nh + 1) * 512],
                                start=(ktp == 0), stop=(ktp == 1), perf_mode=DR,
                            )
                        if ktp == 1:
                            nc.vector.tensor_scalar(
                                out=k_sb[:, dt, :], in0=k_mm[dt], scalar1=1.0 / 16,
                                scalar2=bk_sb[:, dt:dt + 1], op0=Alu.mult,
                                op1=Alu.add)

                # ---- V transposed: vT[m, c] (n stationary, wv moving);
                # v bias folds into bp' on host ----
                for mg in range(KT):  # 2 m-tiles per psum tile
                    mm = ps_qkv.tile([P, N], f32, tag="mm", name=f"v{mg}")
                    for ml in range(2):
                        mt = 2 * mg + ml
                        for ktp in range(2):
                            nc.tensor.matmul(
                                mm[:, ml * 512:(ml + 1) * 512],
                                n_sb[:, 2 * ktp:2 * ktp + 2, mt * P:(mt + 1) * P],
                                wv_sb[:, 2 * ktp:2 * ktp + 2, :],
                                start=(ktp == 0), stop=(ktp == 1), perf_mode=DR,
                            )
                    nc.scalar.activation(
                        out=vT_sb[:, 2 * mg:2 * mg + 2, :],
                        in_=mm.rearrange("p (g c) -> p g c", g=2),
                        func=Act.Identity, bias=zero_sb, scale=1.0 / 16)

            # ---- scores transposed + exp, pipelined per 2 m-tiles ----
            # sT[m, n] = sum_c k[c, m] q[c, n]; exp on ACT -> fp8 eT
            with tc.tile_pool(name="ps_s", bufs=2, space="PSUM") as ps_s:
                for mtp in range(4):
                    s_ps = ps_s.tile([P, 2, N], f32, tag="s", name=f"s{mtp}")
                    for ml in range(2):
                        mt = 2 * mtp + ml
                        for ktp in range(2):
                            for nh in range(2):
                                nc.tensor.matmul(
                                    s_ps[:, ml, nh * 512:(nh + 1) * 512],
                                    k_sb[:, 2 * ktp:2 * ktp + 2, mt * P:(mt + 1) * P],
                                    q_sb[:, 2 * ktp:2 * ktp + 2, nh * 512:(nh + 1) * 512],
                                    start=(ktp == 0), stop=(ktp == 1), perf_mode=DR,
                                )
                    # raw = 16*s_true; exp(SCALE/16 * raw) in [~0.1, ~8]
                    nc.scalar.activation(out=eT_sb[:, 2 * mtp:2 * mtp + 2, :],
                                         in_=s_ps, func=Act.Exp,
                                         bias=zero_sb, scale=SCALE / 16)

            # xbp = x + bp' (residual + folded proj/v bias) on DVE slack
            for dt in range(KT):
                nc.vector.tensor_scalar(
                    out=xbp_sb[:, dt, :], in0=x_sb[:, dt, :],
                    scalar1=bp_sb[:, dt:dt + 1], scalar2=None, op0=Alu.add)

            with tc.tile_pool(name="ps_av", bufs=3, space="PSUM") as ps_av:
                # ---- softmax denominators, broadcast to all partitions;
                # the sum tile shares the attnV pool so no pool seam ----
                sum_ps = ps_av.tile([P, N], f32, tag="mm", name="sum")
                for mtp in range(4):
                    for nh in range(2):
                        nc.tensor.matmul(
                            sum_ps[:, nh * 512:(nh + 1) * 512],
                            ones_sb[:],
                            eT_sb[:, 2 * mtp:2 * mtp + 2, nh * 512:(nh + 1) * 512],
                            start=(mtp == 0), stop=(mtp == 3), perf_mode=DR,
                        )
                nc.vector.reciprocal_approx_fast(out=rsr_sb, in_=sum_ps)

                # ---- out[c, n] = (sum_m vT[m,c] eT[m,n]) / rowsum[n] ----
                for ct in range(KT):
                    mm = ps_av.tile([P, N], f32, tag="mm", name=f"av{ct}")
                    for mtp in range(4):
                        for nh in range(2):
                            nc.tensor.matmul(
                                mm[:, nh * 512:(nh + 1) * 512],
                                vT_sb[:, 2 * mtp:2 * mtp + 2, ct * P:(ct + 1) * P],
                                eT_sb[:, 2 * mtp:2 * mtp + 2, nh * 512:(nh + 1) * 512],
                                start=(mtp == 0), stop=(mtp == 3), perf_mode=DR,
                            )
                    # o8 = raw * rsr = 4*attnout_true (DVE)
                    nc.vector.tensor_tensor(out=o_sb[:, ct, :], in0=mm,
                                            in1=rsr_sb, op=Alu.mult)

            with tc.tile_pool(name="ps_pr", bufs=2, space="PSUM") as ps_pr:
                # ---- final projection + residual, stream out in halves ----
                for dt in range(KT):
                    mm = ps_pr.tile([P, N], f32, tag="mm", name=f"p{dt}")
                    for ktp in range(2):
                        for nh in range(2):
                            nc.tensor.matmul(
                                mm[:, nh * 512:(nh + 1) * 512],
                                wp_sb[:, 2 * ktp:2 * ktp + 2, dt * P:(dt + 1) * P],
                                o_sb[:, 2 * ktp:2 * ktp + 2, nh * 512:(nh + 1) * 512],
                                start=(ktp == 0), stop=(ktp == 1), perf_mode=DR,
                            )
                    for h in range(2):
                        y_sb = work.tile([P, 512], bf16, tag="y")
                        # y = raw/256 + (x + bp')
                        nc.vector.scalar_tensor_tensor(
                            out=y_sb, in0=mm[:, h * 512:(h + 1) * 512],
                            scalar=1.0 / 256,
                            in1=xbp_sb[:, dt, h * 512:(h + 1) * 512],
                            op0=Alu.mult, op1=Alu.add)
                        yq = [nc.scalar, nc.sync][(2 * dt + h) % 2]
                        yq.dma_start(
                            out=y_d[dt * P:(dt + 1) * P, h * 512:(h + 1) * 512],
                            in_=y_sb)

    nc.finalize()
    return nc


def _get_nc():
    if "nc" not in _CACHE:
        _CACHE["nc"] = _build_bass()
    return _CACHE["nc"]


def _make_in_maps(x, gn_w, gn_b, q_w, q_b, k_w, k_b, v_w, v_b, p_w, p_b):
    import ml_dtypes
    f8 = ml_dtypes.float8_e4m3
    bf = ml_dtypes.bfloat16
    x = np.asarray(x, np.float32)
    B = x.shape[0]
    assert x.shape == (B, CH, 32, 32) and B == NCORES

    def pc(vec):  # [512] -> [128, 4] with c = t*128 + p
        return np.asarray(vec, np.float32).reshape(KT, P).T

    def w8(w):  # [Cout, Cin] -> fp8 [P, KT, Cout] of 64*w.T
        wt = np.asarray(w, np.float32).T * 64.0  # [Cin, Cout]
        return np.ascontiguousarray(
            wt.reshape(KT, P, CH).transpose(1, 0, 2).astype(f8))

    bp_fold = np.asarray(p_w, np.float32) @ np.asarray(v_b, np.float32) \
        + np.asarray(p_b, np.float32)
    avg = np.kron(np.eye(2, dtype=np.float32),
                  np.full((64, 64), 1.0 / 64, np.float32))
    consts = np.concatenate(
        [pc(4.0 * np.asarray(q_b)), pc(4.0 * np.asarray(k_b)),
         pc(gn_w), pc(gn_b), pc(bp_fold), avg], axis=1
    )
    shared = {
        "wq8": w8(q_w),
        "wk8": w8(k_w),
        "wv8": w8(v_w),
        "wp8": w8(p_w),
        "consts": np.ascontiguousarray(consts),
    }
    return [
        dict(shared, x=np.ascontiguousarray(x[b].reshape(CH, N).astype(bf)))
        for b in range(B)
    ]


def _run(in_maps, **kwargs):
    from concourse.bass_utils import run_bass_kernel_spmd
    return run_bass_kernel_spmd(_get_nc(), in_maps, core_ids=list(range(NCORES)), **kwargs)


def kernel(**inputs):
    in_maps = _make_in_maps(**inputs)
    res = _run(in_maps)
    out = np.stack([np.asarray(r["y"], dtype=np.float32).reshape(CH, 32, 32)
                    for r in res.results], axis=0)
    return out.astype(np.float32)


# revision 17
# speedup vs baseline: 1.1801x; 1.1801x over previous
# Trainium2 Bass kernel for nn_AttentionBlock (GroupNorm + single-head
# self-attention over 32x32 spatial, C=512) — data-parallel over batch:
# 8 batch elements -> 8 NeuronCores, weights replicated.
#
# fp8(e4m3) DoubleRow design: all big matmuls run as fp8 DoubleRow
# (256-deep contraction per instruction, ~259ns/MM sustained), scores
# are computed transposed (k stationary) so no PE transposes are
# needed, softmax row-sums come from a ones-matrix matmul broadcast to
# all partitions, and the softmax normalization is folded into the
# attn@V eviction. Scale bookkeeping: weights x64 on host, activations
# stored x4 in fp8, descale factors folded into the PSUM evictions.
# x and y travel as bf16 (halves DMA bytes; residual precision loss
# ~2e-3 rel, well under the 2e-2 gate). Evictions are split ACT/DVE to
# balance engines; GpSimd only runs a DMA queue (its compute path is
# ~25x slower than DVE and contends for DVE's SBUF port). Dummy fp8
# matmuls bridge PE-idle windows in the GroupNorm phase so the HAM
# clock gate stays at full rate when the projections start.
import numpy as np

CH = 512          # channels
N = 1024          # spatial H*W = 32*32
P = 128           # SBUF partitions
KT = CH // P      # 4 channel tiles
MT = N // P       # 8 spatial tiles (keys)
GROUPS = 8        # groupnorm groups (64 channels each)
EPS = 1e-5
SCALE = 1.0 / np.sqrt(CH)
NCORES = 8

_CACHE = {}


def _build_bass():
    import concourse.bacc as bacc
    import concourse.tile as tile
    from concourse import mybir

    f32 = mybir.dt.float32
    f8 = mybir.dt.float8e4
    bf16 = mybir.dt.bfloat16
    Act = mybir.ActivationFunctionType
    Alu = mybir.AluOpType
    DR = mybir.MatmulPerfMode.DoubleRow

    nc = bacc.Bacc("TRN2")

    x_d = nc.dram_tensor("x", [CH, N], bf16, kind="ExternalInput")
    wq_d = nc.dram_tensor("wq8", [P, KT, CH], f8, kind="ExternalInput")
    wk_d = nc.dram_tensor("wk8", [P, KT, CH], f8, kind="ExternalInput")
    wv_d = nc.dram_tensor("wv8", [P, KT, CH], f8, kind="ExternalInput")
    wp_d = nc.dram_tensor("wp8", [P, KT, CH], f8, kind="ExternalInput")
    # packed per-channel vectors (cols 0..19 = 4*bq|4*bk|gnw|gnb|bp')
    # followed by the 128x128 group-averaging matrix (cols 20..147)
    con_d = nc.dram_tensor("consts", [P, 148], f32, kind="ExternalInput")
    y_d = nc.dram_tensor("y", [CH, N], bf16, kind="ExternalOutput")

    with tile.TileContext(nc) as tc:
        with (
            tc.tile_pool(name="persist", bufs=1) as persist,
            tc.tile_pool(name="work", bufs=2) as work,
            tc.tile_pool(name="small", bufs=2) as small,
        ):
            # ---- persistent SBUF tensors ----
            x_sb = persist.tile([P, KT, N], bf16, tag="x")
            n_sb = persist.tile([P, KT, N], f8, tag="n")
            q_sb = persist.tile([P, KT, N], f8, tag="q")
            k_sb = persist.tile([P, KT, N], f8, tag="k")
            vT_sb = persist.tile([P, MT, CH], f8, tag="vT")
            eT_sb = persist.tile([P, MT, N], f8, tag="eT")
            o_sb = persist.tile([P, KT, N], f8, tag="o")
            rsr_sb = persist.tile([P, N], f32, tag="rsr")
            xbp_sb = persist.tile([P, KT, N], f32, tag="xbp")
            wq_sb = persist.tile([P, KT, CH], f8, tag="wq")
            wk_sb = persist.tile([P, KT, CH], f8, tag="wk")
            wv_sb = persist.tile([P, KT, CH], f8, tag="wv")
            wp_sb = persist.tile([P, KT, CH], f8, tag="wp")
            con_sb = persist.tile([P, 148], f32, tag="consts")
            vec_sb = con_sb[:, 0:20]
            avg_sb = persist.tile([P, P], bf16, tag="avg")
            ones_sb = persist.tile([P, 2, P], f8, tag="ones")
            warm_sb = persist.tile([P, 2, P], f8, tag="warm")
            scr_sb = persist.tile([P, N], f32, tag="scr")
            zero_sb = persist.tile([P, 1], f32, tag="zero")
            eps_sb = persist.tile([P, 1], f32, tag="eps")
            dummy_sb = persist.tile([P, 1], f32, tag="dummy")
            bq_sb = vec_sb[:, 0:4]     # 4*q_b
            bk_sb = vec_sb[:, 4:8]     # 4*k_b
            gnw_sb = vec_sb[:, 8:12]
            gnb_sb = vec_sb[:, 12:16]
            bp_sb = vec_sb[:, 16:20]   # p_w @ v_b + p_b

            # constants + ACT sqrt-table preload while DMAs stream
            nc.vector.memset(zero_sb, 0.0)
            nc.vector.memset(eps_sb, EPS)
            nc.vector.memset(dummy_sb, 1.0)
            nc.vector.memset(ones_sb, 1.0)
            nc.vector.memset(warm_sb, 1.0)
            nc.scalar.activation(out=dummy_sb, in_=dummy_sb, func=Act.Sqrt,
                                 bias=zero_sb, scale=1.0)

            # ---- loads: x on the sync queue, weights in parallel on the
            # gpsimd queue (gpsimd does no compute in this kernel) ----
            for h in range(2):
                nc.sync.dma_start(
                    out=x_sb[:, 0, h * 512:(h + 1) * 512],
                    in_=x_d[0:P, h * 512:(h + 1) * 512])
                nc.scalar.dma_start(
                    out=x_sb[:, 1, h * 512:(h + 1) * 512],
                    in_=x_d[P:2 * P, h * 512:(h + 1) * 512])
            nc.sync.dma_start(out=x_sb[:, 2, :], in_=x_d[2 * P:3 * P, :])
            nc.scalar.dma_start(out=x_sb[:, 3, :], in_=x_d[3 * P:4 * P, :])
            nc.gpsimd.dma_start(out=con_sb[:], in_=con_d[:])
            nc.gpsimd.dma_start(out=wq_sb[:], in_=wq_d[:])
            nc.gpsimd.dma_start(out=wk_sb[:], in_=wk_d[:])
            nc.gpsimd.dma_start(out=wv_sb[:], in_=wv_d[:])
            nc.gpsimd.dma_start(out=wp_sb[:], in_=wp_d[:])

            with tc.tile_pool(name="ps_g", bufs=2, space="PSUM") as ps_g:
                # PE warmth: cheap small fp8 DR matmuls into a scratch bank
                warm_ps = ps_g.tile([P, P], f32, tag="warmps")

                def warm(k):
                    for _ in range(k):
                        nc.tensor.matmul(warm_ps, ones_sb[:], warm_sb[:],
                                         start=True, stop=True, perf_mode=DR)

                warm(28)

                # cast the group-averaging matrix to bf16 (1-pass matmuls)
                nc.vector.tensor_copy(avg_sb, con_sb[:, 20:148])

                # ---- GroupNorm, two batches of 2 channel-tiles each:
                # bn_stats (DVE) -> one bf16 group matmul per batch ->
                # short DVE chain -> per-kt affine on ACT -> n in fp8 ----
                for b in range(2):
                    st = small.tile([P, 4], bf16, tag="st")  # m0|m1|E0|E1
                    for j in range(2):
                        kt = 2 * b + j
                        bstats = small.tile([P, 2, 6], f32, tag="bstats")
                        mv = small.tile([P, 2], f32, tag="mv")
                        nc.vector.bn_stats(out=bstats[:, 0, :], in_=x_sb[:, kt, 0:512])
                        nc.vector.bn_stats(out=bstats[:, 1, :], in_=x_sb[:, kt, 512:1024])
                        nc.vector.bn_aggr(out=mv, in_=bstats)
                        nc.vector.tensor_copy(st[:, j:j + 1], mv[:, 0:1])
                        nc.vector.scalar_tensor_tensor(
                            out=st[:, 2 + j:3 + j], in0=mv[:, 0:1], scalar=mv[:, 0:1],
                            in1=mv[:, 1:2], op0=Alu.mult, op1=Alu.add,
                        )

                    # group aggregate+broadcast for both kt in one matmul
                    b_ps = ps_g.tile([P, 4], f32, tag="gmm")
                    nc.tensor.matmul(b_ps, avg_sb, st, start=True, stop=True)
                    warm(12)
                    bc = small.tile([P, 4], f32, tag="bc")
                    nc.vector.tensor_copy(bc, b_ps)
                    gmean = bc[:, 0:2]
                    msq = small.tile([P, 2], f32, tag="msq")
                    nc.vector.tensor_tensor(out=msq, in0=gmean, in1=gmean,
                                            op=Alu.mult)
                    vneg = small.tile([P, 2], f32, tag="vneg")
                    nc.vector.tensor_tensor(out=vneg, in0=msq, in1=bc[:, 2:4],
                                            op=Alu.subtract)  # mean^2 - E
                    sd = small.tile([P, 2], f32, tag="sd")
                    nc.scalar.activation(out=sd, in_=vneg, func=Act.Sqrt,
                                         bias=eps_sb, scale=-1.0)
                    rstd = small.tile([P, 2], f32, tag="rstd")
                    nc.vector.reciprocal(rstd, sd)
                    gsc = small.tile([P, 2], f32, tag="gsc")
                    nc.vector.tensor_tensor(out=gsc, in0=gnw_sb[:, 2 * b:2 * b + 2],
                                            in1=rstd, op=Alu.mult)
                    mg = small.tile([P, 2], f32, tag="mg")
                    nc.vector.tensor_tensor(out=mg, in0=gmean, in1=gsc,
                                            op=Alu.mult)
                    gshp = small.tile([P, 2], f32, tag="gshp")  # gnb - mean*gsc
                    nc.vector.tensor_tensor(out=gshp,
                                            in0=gnb_sb[:, 2 * b:2 * b + 2],
                                            in1=mg, op=Alu.subtract)
                    for j in range(2):
                        kt = 2 * b + j
                        # n8 = x*gsc + gshp  (ACT, fp8 out)
                        nc.scalar.activation(out=n_sb[:, kt, :],
                                             in_=x_sb[:, kt, :],
                                             func=Act.Identity,
                                             bias=gshp[:, j:j + 1],
                                             scale=gsc[:, j:j + 1])

                warm(12)
                # preload the exp table while the QKV matmuls stream
                nc.scalar.activation(out=dummy_sb, in_=x_sb[:, 0, 0:1], func=Act.Exp,
                                     bias=zero_sb, scale=1.0)

            with tc.tile_pool(name="ps_qkv", bufs=4, space="PSUM") as ps_qkv:
                # ---- Q projection (DoubleRow, weights stationary).
                # ktp-outer: the ktp=0 sweep only needs n8 kt0/kt1, so the
                # PE starts while the GN tail still runs on ACT/DVE. ----
                q_mm = [ps_qkv.tile([P, N], f32, tag="mm", name=f"q{dt}")
                        for dt in range(KT)]
                for ktp in range(2):
                    for dt in range(KT):
                        for nh in range(2):
                            nc.tensor.matmul(
                                q_mm[dt][:, nh * 512:(nh + 1) * 512],
                                wq_sb[:, 2 * ktp:2 * ktp + 2, dt * P:(dt + 1) * P],
                                n_sb[:, 2 * ktp:2 * ktp + 2, nh * 512:(nh + 1) * 512],
                                start=(ktp == 0), stop=(ktp == 1), perf_mode=DR,
                            )
                        if ktp == 1:
                            # q8 = raw/16 + 4*bq  (= 4*q_true), ACT evict
                            nc.scalar.activation(out=q_sb[:, dt, :], in_=q_mm[dt],
                                                 func=Act.Identity,
                                                 bias=bq_sb[:, dt:dt + 1],
                                                 scale=1.0 / 16)

                # ---- K projection: evict on DVE to balance engines ----
                k_mm = [ps_qkv.tile([P, N], f32, tag="mm", name=f"k{dt}")
                        for dt in range(KT)]
                for ktp in range(2):
                    for dt in range(KT):
                        for nh in range(2):
                            nc.tensor.matmul(
                                k_mm[dt][:, nh * 512:(nh + 1) * 512],
                                wk_sb[:, 2 * ktp:2 * ktp + 2, dt * P:(dt + 1) * P],
                                n_sb[:, 2 * ktp:2 * ktp + 2, nh * 512:(nh + 1) * 512],
                                start=(ktp == 0), stop=(ktp == 1), perf_mode=DR,
                            )
                        if ktp == 1:
                            nc.vector.tensor_scalar(
                                out=k_sb[:, dt, :], in0=k_mm[dt], scalar1=1.0 / 16,
                                scalar2=bk_sb[:, dt:dt + 1], op0=Alu.mult,
                                op1=Alu.add)

                # ---- V transposed: vT[m, c] (n stationary, wv moving);
                # v bias folds into bp' on host ----
                for mg in range(KT):  # 2 m-tiles per psum tile
                    mm = ps_qkv.tile([P, N], f32, tag="mm", name=f"v{mg}")
                    for ml in range(2):
                        mt = 2 * mg + ml
                        for ktp in range(2):
                            nc.tensor.matmul(
                                mm[:, ml * 512:(ml + 1) * 512],
                                n_sb[:, 2 * ktp:2 * ktp + 2, mt * P:(mt + 1) * P],
                                wv_sb[:, 2 * ktp:2 * ktp + 2, :],
                                start=(ktp == 0), stop=(ktp == 1), perf_mode=DR,
                            )
                    nc.scalar.activation(
                        out=vT_sb[:, 2 * mg:2 * mg + 2, :],
                        in_=mm.rearrange("p (g c) -> p g c", g=2),
                        func=Act.Identity, bias=zero_sb, scale=1.0 / 16)

            # ---- scores transposed + exp, pipelined per 2 m-tiles ----
            # sT[m, n] = sum_c k[c, m] q[c, n]; exp on ACT -> fp8 eT
            with tc.tile_pool(name="ps_s", bufs=2, space="PSUM") as ps_s:
                for mtp in range(4):
                    s_ps = ps_s.tile([P, 2, N], f32, tag="s", name=f"s{mtp}")
                    for ml in range(2):
                        mt = 2 * mtp + ml
                        for ktp in range(2):
                            for nh in range(2):
                                nc.tensor.matmul(
                                    s_ps[:, ml, nh * 512:(nh + 1) * 512],
                                    k_sb[:, 2 * ktp:2 * ktp + 2, mt * P:(mt + 1) * P],
                                    q_sb[:, 2 * ktp:2 * ktp + 2, nh * 512:(nh + 1) * 512],
                                    start=(ktp == 0), stop=(ktp == 1), perf_mode=DR,
                                )
                    # raw = 16*s_true; exp(SCALE/16 * raw) in [~0.1, ~8]
                    nc.scalar.activation(out=eT_sb[:, 2 * mtp:2 * mtp + 2, :],
                                         in_=s_ps, func=Act.Exp,
                                         bias=zero_sb, scale=SCALE / 16)

            # xbp = x + bp' (residual + folded proj/v bias) on DVE slack
            for dt in range(KT):
                nc.vector.tensor_scalar(
                    out=xbp_sb[:, dt, :], in0=x_sb[:, dt, :],
                    scalar1=bp_sb[:, dt:dt + 1], scalar2=None, op0=Alu.add)

            with tc.tile_pool(name="ps_av", bufs=3, space="PSUM") as ps_av:
                # ---- softmax denominators, broadcast to all partitions;
                # the sum tile shares the attnV pool so no pool seam ----
                sum_ps = ps_av.tile([P, N], f32, tag="mm", name="sum")
                for mtp in range(4):
                    for nh in range(2):
                        nc.tensor.matmul(
                            sum_ps[:, nh * 512:(nh + 1) * 512],
                            ones_sb[:],
                            eT_sb[:, 2 * mtp:2 * mtp + 2, nh * 512:(nh + 1) * 512],
                            start=(mtp == 0), stop=(mtp == 3), perf_mode=DR,
                        )
                nc.vector.reciprocal_approx_fast(out=rsr_sb, in_=sum_ps)

                # ---- out[c, n] = (sum_m vT[m,c] eT[m,n]) / rowsum[n] ----
                for ct in range(KT):
                    mm = ps_av.tile([P, N], f32, tag="mm", name=f"av{ct}")
                    for mtp in range(4):
                        for nh in range(2):
                            nc.tensor.matmul(
                                mm[:, nh * 512:(nh + 1) * 512],
                                vT_sb[:, 2 * mtp:2 * mtp + 2, ct * P:(ct + 1) * P],
                                eT_sb[:, 2 * mtp:2 * mtp + 2, nh * 512:(nh + 1) * 512],
                                start=(mtp == 0), stop=(mtp == 3), perf_mode=DR,
                            )
                    # o8 = raw * rsr = 4*attnout_true (DVE)
                    nc.vector.tensor_tensor(out=o_sb[:, ct, :], in0=mm,
                                            in1=rsr_sb, op=Alu.mult)

            with tc.tile_pool(name="ps_pr", bufs=2, space="PSUM") as ps_pr:
                # ---- final projection + residual, stream out in halves ----
                for dt in range(KT):
                    mm = ps_pr.tile([P, N], f32, tag="mm", name=f"p{dt}")
                    for ktp in range(2):
                        for nh in range(2):
                            nc.tensor.matmul(
                                mm[:, nh * 512:(nh + 1) * 512],
                                wp_sb[:, 2 * ktp:2 * ktp + 2, dt * P:(dt + 1) * P],
                                o_sb[:, 2 * ktp:2 * ktp + 2, nh * 512:(nh + 1) * 512],
                                start=(ktp == 0), stop=(ktp == 1), perf_mode=DR,
                            )
                    for h in range(2):
                        y_sb = work.tile([P, 512], bf16, tag="y")
                        # y = raw/256 + (x + bp')
                        nc.vector.scalar_tensor_tensor(
                            out=y_sb, in0=mm[:, h * 512:(h + 1) * 512],
                            scalar=1.0 / 256,
                            in1=xbp_sb[:, dt, h * 512:(h + 1) * 512],
                            op0=Alu.mult, op1=Alu.add)
                        yq = [nc.scalar, nc.sync][(2 * dt + h) % 2]
                        yq.dma_start(
                            out=y_d[dt * P:(dt + 1) * P, h * 512:(h + 1) * 512],
                            in_=y_sb)

    nc.finalize()
    return nc


def _get_nc():
    if "nc" not in _CACHE:
        _CACHE["nc"] = _build_bass()
    return _CACHE["nc"]


def _make_in_maps(x, gn_w, gn_b, q_w, q_b, k_w, k_b, v_w, v_b, p_w, p_b):
    import ml_dtypes
    f8 = ml_dtypes.float8_e4m3
    bf = ml_dtypes.bfloat16
    x = np.asarray(x, np.float32)
    B = x.shape[0]
    assert x.shape == (B, CH, 32, 32) and B == NCORES

    def pc(vec):  # [512] -> [128, 4] with c = t*128 + p
        return np.asarray(vec, np.float32).reshape(KT, P).T

    def w8(w):  # [Cout, Cin] -> fp8 [P, KT, Cout] of 64*w.T
        wt = np.asarray(w, np.float32).T * 64.0  # [Cin, Cout]
        return np.ascontiguousarray(
            wt.reshape(KT, P, CH).transpose(1, 0, 2).astype(f8))

    bp_fold = np.asarray(p_w, np.float32) @ np.asarray(v_b, np.float32) \
        + np.asarray(p_b, np.float32)
    avg = np.kron(np.eye(2, dtype=np.float32),
                  np.full((64, 64), 1.0 / 64, np.float32))
    consts = np.concatenate(
        [pc(4.0 * np.asarray(q_b)), pc(4.0 * np.asarray(k_b)),
         pc(gn_w), pc(gn_b), pc(bp_fold), avg], axis=1
    )
    shared = {
        "wq8": w8(q_w),
        "wk8": w8(k_w),
        "wv8": w8(v_w),
        "wp8": w8(p_w),
        "consts": np.ascontiguousarray(consts),
    }
    return [
        dict(shared, x=np.ascontiguousarray(x[b].reshape(CH, N).astype(bf)))
        for b in range(B)
    ]


def _run(in_maps, **kwargs):
    from concourse.bass_utils import run_bass_kernel_spmd
    return run_bass_kernel_spmd(_get_nc(), in_maps, core_ids=list(range(NCORES)), **kwargs)


def kernel(**inputs):
    in_maps = _make_in_maps(**inputs)
    res = _run(in_maps)
    out = np.stack([np.asarray(r["y"], dtype=np.float32).reshape(CH, 32, 32)
                    for r in res.results], axis=0)
    return out.astype(np.float32)
